# revision 75
# baseline (speedup 1.0000x reference)
"""BDH linear-attention TRN2 kernel v3 — fp8 DoubleRow matmuls, data-parallel
over batch on 8 cores.

Per-core program (core b handles batch b):
  A. LayerNorm -> xn (bf16); PE-transpose -> xnT fp8 [128,6,1024]; gates and
     residue: [rg|wg|res]-augmented DR matmul columns over xnT ([32,512]
     psums; rows 0-15 reduce to the read/write gate logits), residue via a
     bf16 stt accumulate (fp8 logits are too noisy for the residual blend).
  B. k/v projections as fp8 DoubleRow matmuls vs host-packed weights; hub
     feature map (max, *2^42) -> bf16, bits>>1 sqrt, t*s -> khat fp8, with
     passes split across ACT/DVE/Pool per the tuned STRAT table; per-head
     state via DoubleRow over token-tile pairs; k-normalization via 32-wide
     ones-column DR sums.  AllReduce in three chunks (heads 0-3 + write
     gates at h==3, heads 4-5 at h==5, heads 6-7 at h==7) so the D-phase
     can start as halves land; D-phase for heads 0-3 is emitted mid-B.
  C. q projection (fp8 DR) + feature map -> qhatT fp8 [128,24,1024].
  D. m' = memT64 + (boa*wsum_h)*st (fp8); W'_h = a_h*(m'^T @ wo64) -> Wp fp8.
  E. psum = qhatT^T @ Wp (fp8 DR); out = residue/DESC*psum + (1-residue)*x.

Scales: w_in*8 (host, fp8), feature map absorbs 8^1.5; memT*64, w_out*64
(host); final descale 8^1.5*64*64 folded into the residue multiplier.
"""
import numpy as np
import ml_dtypes

import concourse.mybir as mybir
import concourse.tile as tile
from concourse import bacc
from concourse.masks import make_identity
from concourse.bass_utils import run_bass_kernel_spmd

F32 = mybir.dt.float32
BF16 = mybir.dt.bfloat16
FP8 = mybir.dt.float8e4
AF = mybir.ActivationFunctionType
OP = mybir.AluOpType
DR = mybir.MatmulPerfMode.DoubleRow
U16 = mybir.dt.uint16
# bit-hack sqrt: s = bitcast(bits(t*2^42) >> 1) = C*sqrt(t), C folded into
# the output descale (k-side cancels in the khat normalization)
SQ_PRE = 2.0 ** 42
SQ_C = 0.7199236             # k = t*s = SQ_C * t^1.5 (+-4.5%)

B, N, D, H = 8, 1024, 768, 8
S = 3072
HD = 384
NT = N // 128        # 8 token tiles
KC = D // 128        # 6 contraction chunks
SC = S // 128        # 24 sparse chunks
EPS = 1e-6
LN_EPS = 1e-5
PERSIST = 0.95
N_CORES = 8

SW = 8.0                     # host prescale of w_in
KTHR = SW * EPS              # feature-map clamp threshold (scaled)
MSC = 64.0                   # host prescale of memT / w_out
OUT_DESCALE = 0.7199236 * SW ** 1.5 * MSC * MSC
BOA = (1.0 - PERSIST) / (N_CORES * N_CORES * SW * PERSIST)

CC_HALF = 4 * 3 * 128 * HD   # bf16 elements per half (4 heads)
CC_LEN_A = CC_HALF + 8
CC_LEN_B = CC_HALF
HBLK = 3 * 128 * HD          # per-head cc elements

NP_FP8 = ml_dtypes.float8_e4m3
NP_BF16 = ml_dtypes.bfloat16

# engine-assignment strategy knobs (tuned offline with perf_probe.py;
# the defaults below are the swept optimum for this shape)
STRAT = {
    "kmul": 2,    # khat fmap mul: 0 all-pool, 1 25% dve, 2 50% dve, 3 all dve
    "qmul": 0,    # qhat fmap mul: 0 baseline 50% dve, 1 all pool, 2 all dve
    "kp1": 0,     # k relu pass:   0 50/50 act/dve, 1 all act, 2 all dve
    "qp1": 1,     # q relu pass:   0 75% act, 1 all act, 2 50/50
    "vcopy": 0,   # v copies:      0 all act, 1 50/50, 2 all dve
    "lnxn": 0,    # LN normalize:  0 split, 1 all act
    "wp": 0,      # Wp epilogue:   0 split, 1 all act, 2 all dve
}



def build_program(ln_trivial, b_in_zero, b_out_zero, single_core=False):
    nc = bacc.Bacc("TRN2", target_bir_lowering=False, debug=False,
                   num_devices=1 if single_core else N_CORES)

    x_d = nc.dram_tensor("x", [N, D], F32, kind="ExternalInput")
    wkv_d = nc.dram_tensor("wkv", [128, 96 * 384], FP8, kind="ExternalInput")
    wq_d = nc.dram_tensor("wq", [128, 144 * 128], FP8, kind="ExternalInput")
    wo_d = nc.dram_tensor("wo", [128, 48 * 384], FP8, kind="ExternalInput")
    mem_d = nc.dram_tensor("memT64", [128, 24 * 384], BF16, kind="ExternalInput")
    wg32_d = nc.dram_tensor("wg32", [128, KC * 32], FP8, kind="ExternalInput")
    gb32_d = nc.dram_tensor("gb32", [32], F32, kind="ExternalInput")
    wres16_d = nc.dram_tensor("wres16", [D], BF16, kind="ExternalInput")
    b_res_d = nc.dram_tensor("b_res", [1], F32, kind="ExternalInput")
    if not ln_trivial:
        ln_g_d = nc.dram_tensor("ln_g", [D], F32, kind="ExternalInput")
        ln_b_d = nc.dram_tensor("ln_b", [D], F32, kind="ExternalInput")
    if not b_in_zero:
        bq8_d = nc.dram_tensor("bq8", [128, SC], F32, kind="ExternalInput")
        bkv8_d = nc.dram_tensor("bkv8", [16, HD], F32, kind="ExternalInput")
    if not b_out_zero:
        b_out_d = nc.dram_tensor("b_out", [D], F32, kind="ExternalInput")
    out_d = nc.dram_tensor("out", [N, D], F32, kind="ExternalOutput")

    with tile.TileContext(nc) as tc:
        with (
            tc.tile_pool(name="const", bufs=1) as const,
            tc.tile_pool(name="resid", bufs=1) as resid,
            tc.tile_pool(name="wtop", bufs=1) as wtop,
            tc.tile_pool(name="scrD", bufs=1) as scrD,
            tc.tile_pool(name="ccdram", bufs=1, space="DRAM") as ccdram,
        ):

            # ---------------- first x tiles at the head of the DMA queue ---
            x_sb = [wtop.tile([128, D], F32, name=f"x{t}") for t in range(NT)]
            for t in range(4):
                nc.sync.dma_start(x_sb[t][:], x_d[t * 128:(t + 1) * 128, :])

            # ---------------- constants ------------------------------------
            ident = const.tile([128, 128], BF16)
            make_identity(nc, ident[:])
            ones8t = const.tile([128, NT, 32], FP8)
            nc.vector.memset(ones8t[:], 1.0)
            lneps_col = const.tile([128, 1], F32)
            nc.vector.memset(lneps_col[:], LN_EPS)
            # warm the sqrt ACT table (includes copy/relu) while DMAs land
            warm = const.tile([128, 1], F32)
            nc.scalar.activation(warm[:], lneps_col[:], AF.Sqrt)
            wres_b = const.tile([128, D], BF16)
            nc.sync.dma_start(wres_b[:], wres16_d.ap().partition_broadcast(128))
            bres_b = const.tile([128, 1], F32)
            nc.sync.dma_start(bres_b[:], b_res_d.ap().partition_broadcast(128))
            wg32_sb = const.tile([128, KC, 32], FP8)
            nc.sync.dma_start(wg32_sb[:],
                              wg32_d.ap().rearrange("p (c g) -> p c g", c=KC))
            gb32 = const.tile([32, 1], F32)
            nc.sync.dma_start(gb32[:], gb32_d.ap().rearrange("(p f) -> p f",
                                                             f=1))
            if not ln_trivial:
                lng_b = const.tile([128, D], F32)
                nc.gpsimd.dma_start(lng_b[:], ln_g_d.ap().partition_broadcast(128))
                lnb_b = const.tile([128, D], F32)
                nc.gpsimd.dma_start(lnb_b[:], ln_b_d.ap().partition_broadcast(128))
            if not b_in_zero:
                bq_sb = const.tile([128, SC], F32)
                nc.sync.dma_start(bq_sb[:], bq8_d[:, :])
                bkv_sb = const.tile([128, 16, HD], F32)
                nc.sync.dma_start(bkv_sb[:], bkv8_d.ap().partition_broadcast(128))
            if not b_out_zero:
                bout_b = const.tile([128, D], F32)
                nc.sync.dma_start(bout_b[:], b_out_d.ap().partition_broadcast(128))

            residue = resid.tile([128, NT], F32)
            rdiv = resid.tile([128, NT], F32)
            onemr = resid.tile([128, NT], F32)
            rlogs = resid.tile([128, NT], F32)
            gates32 = resid.tile([32, 1], F32)
            xnT = wtop.tile([128, KC, N], FP8)
            for t in range(4, NT):
                nc.sync.dma_start(x_sb[t][:], x_d[t * 128:(t + 1) * 128, :])
            qhatT = wtop.tile([128, SC, N], FP8)
            WpT = wtop.tile([128, SC, D], FP8)
            wo_sb = wtop.tile([128, 48, 384], FP8)
            stA_t = scrD.tile([128, 12, 384], BF16)
            mA_t = scrD.tile([128, 12, 384], FP8)
            memA_t = scrD.tile([128, 12, 384], BF16)

            cc_in_a = ccdram.tile([CC_LEN_A], BF16)
            cc_in_b1 = ccdram.tile([2 * HBLK], BF16)
            cc_in_b2 = ccdram.tile([2 * HBLK], BF16)
            cc_out_a = ccdram.tile([CC_LEN_A], BF16,
                                   addr_space="Local" if single_core else "Shared")
            cc_out_b1 = ccdram.tile([2 * HBLK], BF16,
                                    addr_space="Local" if single_core else "Shared")
            cc_out_b2 = ccdram.tile([2 * HBLK], BF16,
                                    addr_space="Local" if single_core else "Shared")
            ab_dram = ccdram.tile([16], F32)
            rr_dram = ccdram.tile([N], F32)
            rg_dram = ccdram.tile([8], F32)
            zr_dram = ccdram.tile([H * HD], BF16)

            def do_collective(cin, cout, clen):
                if single_core:
                    nfull = (clen // 128) * 128
                    nc.sync.dma_start(
                        cout[0:nfull].rearrange("(p f) -> p f", p=128),
                        cin[0:nfull].rearrange("(p f) -> p f", p=128))
                    if clen > nfull:
                        nc.sync.dma_start(cout[nfull:clen],
                                          cin[nfull:clen])
                else:
                    nc.gpsimd.collective_compute(
                        "AllReduce", OP.add,
                        replica_groups=[list(range(N_CORES))],
                        ins=[cin.opt()], outs=[cout.opt()])

            def d_prep():
                wsum16 = scrD.tile([1, 8], BF16)
                nc.sync.dma_start(wsum16[:], cc_out_a[CC_HALF:CC_HALF + 8])
                wsum = scrD.tile([1, 8], F32)
                nc.vector.tensor_copy(wsum[:], wsum16[:])
                rgrow = scrD.tile([1, 8], F32)
                nc.sync.dma_start(rgrow[:], rg_dram[:].partition_broadcast(1))
                ab = scrD.tile([1, 16], F32)
                nc.vector.tensor_scalar_mul(ab[:, 0:8], rgrow[:],
                                            PERSIST)
                nc.vector.tensor_scalar_mul(ab[:, 8:16], wsum[:], BOA)
                nc.sync.dma_start(ab_dram[:], ab[:].opt())
                absb = scrD.tile([128, 16], F32)
                nc.sync.dma_start(absb[:], ab_dram[:].partition_broadcast(128))
                return absb

            def d_read(hs, st_t):
                if hs == 0:
                    nc.sync.dma_start(
                        st_t[:],
                        cc_out_a[0:CC_HALF].rearrange("(a p m) -> p a m",
                                                      a=12, p=128))
                    return
                for i, cout in enumerate((cc_out_b1, cc_out_b2)):
                    nc.sync.dma_start(
                        st_t[:, 6 * i:6 * i + 6, :],
                        cout[0:2 * HBLK].rearrange("(a p m) -> p a m",
                                                   a=6, p=128))

            def d_piece(hs, hh, absb, st_t, m_t, mem_t, ps_wp):
                h = hs * 4 + hh
                nc.vector.scalar_tensor_tensor(
                    m_t[:, hh * 3:hh * 3 + 3, :],
                    st_t[:, hh * 3:hh * 3 + 3, :],
                    absb[:, 8 + h:9 + h],
                    mem_t[:, hh * 3:hh * 3 + 3, :],
                    OP.mult, OP.add)
                if True:
                    pass
                    for dc in range(3):
                        for jb in range(2):
                            pwp = ps_wp.tile([128, HD], F32, tag="wp")
                            nc.tensor.matmul(
                                pwp[:],
                                m_t[:, hh * 3:hh * 3 + 2,
                                    dc * 128:(dc + 1) * 128],
                                wo_sb[:, h * 6 + jb * 3:
                                      h * 6 + jb * 3 + 2, :],
                                start=True, stop=False, perf_mode=DR)
                            nc.tensor.matmul(
                                pwp[:],
                                m_t[:, hh * 3 + 2, dc * 128:(dc + 1) * 128],
                                wo_sb[:, h * 6 + jb * 3 + 2, :],
                                start=False, stop=True)
                            dst = WpT[:, h * 3 + dc, jb * 384:(jb + 1) * 384]
                            if STRAT["wp"] == 1 or (
                                    STRAT["wp"] == 0 and (dc + jb) % 2 == 0):
                                nc.scalar.mul(dst, pwp[:], absb[:, h:h + 1])
                            else:
                                nc.vector.tensor_scalar_mul(
                                    dst, pwp[:], absb[:, h:h + 1])

            def q_block2(sc, scr, psq):
                t16 = scr.tile([128, 2, 512], BF16, tag="t16")
                for nh in range(2):
                    pq = psq.tile([128, 512], F32, tag="q")
                    for i in range(3):
                        nc.tensor.matmul(
                            pq[:],
                            wq_sb[:, sc * 6 + 2 * i:sc * 6 + 2 * i + 2, :],
                            xnT[:, 2 * i:2 * i + 2, nh * 512:(nh + 1) * 512],
                            start=(i == 0), stop=(i == 2), perf_mode=DR)
                    _qact = (STRAT["qp1"] == 1 or
                             (STRAT["qp1"] == 0
                              and ((2 * sc + nh) % 2 == 0 or sc >= 16)) or
                             (STRAT["qp1"] == 2 and (2 * sc + nh) % 2 == 0))
                    if _qact and b_in_zero:
                        nc.scalar.activation(t16[:, nh, :], pq[:], AF.Relu,
                                             scale=SQ_PRE)
                    else:
                        with nc.allow_low_precision(reason="fmap"):
                            if not b_in_zero:
                                nc.vector.tensor_scalar(t16[:, nh, :], pq[:],
                                                        bq_sb[:, sc:sc + 1],
                                                        KTHR, OP.add, OP.max)
                                nc.vector.tensor_scalar_mul(
                                    t16[:, nh, :], t16[:, nh, :], SQ_PRE)
                            else:
                                nc.vector.tensor_scalar(t16[:, nh, :], pq[:],
                                                        KTHR, SQ_PRE,
                                                        OP.max, OP.mult)
                s16 = scr.tile([128, 2, 512], BF16, tag="s16")
                nc.vector.tensor_scalar(
                    s16[:].bitcast(U16), t16[:].bitcast(U16),
                    1, None, OP.logical_shift_right)
                qmv = STRAT["qmul"]
                me = (nc.vector if qmv == 2 or (
                    qmv == 0 and (sc % 4 == 1 or sc >= 16)) or (
                    qmv == 4 and sc % 3 == 1) else nc.gpsimd)
                me.tensor_mul(qhatT[:, sc, :], t16[:], s16[:])

            # ================= phases A & B ================================
            with (
                tc.tile_pool(name="wkvp", bufs=1) as wkvp,
                tc.tile_pool(name="kvp", bufs=3) as kvp,
                tc.tile_pool(name="scrB", bufs=2) as scrB,
                tc.tile_pool(name="ps_kv", bufs=3, space="PSUM") as ps_kv,
            ):
                wkv_sb = wkvp.tile([128, 96, 384], FP8)
                wkv_r = wkv_d.ap().rearrange("p (a m) -> p a m", a=96)
                for hw in range(H):
                    nc.sync.dma_start(
                        wkv_sb[:, hw * 12:(hw + 1) * 12, :],
                        wkv_r[:, hw * 12:(hw + 1) * 12, :])
                wq_sb = wkvp.tile([128, 144, 128], FP8)
                wq_r = wq_d.ap().rearrange("p (a m) -> p a m", a=144)
                mem_r = mem_d.ap().rearrange("p (a m) -> p a m", a=24)

                wo_r = wo_d.ap().rearrange("p (a m) -> p a m", a=48)

                def late_loads(h):
                    ll = STRAT.get("ll", 0)
                    if ll == 0:
                        if h == 0:
                            nc.sync.dma_start(wq_sb[:, 0:72, :],
                                              wq_r[:, 0:72, :])
                            nc.sync.dma_start(wq_sb[:, 72:144, :],
                                              wq_r[:, 72:144, :])
                        elif h == 1:
                            nc.sync.dma_start(memA_t[:], mem_r[:, 0:12, :])
                        elif h == 2:
                            nc.sync.dma_start(wo_sb[:, 0:24, :],
                                              wo_r[:, 0:24, :])
                            nc.sync.dma_start(wo_sb[:, 24:48, :],
                                              wo_r[:, 24:48, :])
                    elif ll == 1:
                        if h == 0:
                            nc.sync.dma_start(wq_sb[:, 0:72, :],
                                              wq_r[:, 0:72, :])
                        elif h == 1:
                            nc.sync.dma_start(wq_sb[:, 72:144, :],
                                              wq_r[:, 72:144, :])
                        elif h == 2:
                            nc.sync.dma_start(memA_t[:], mem_r[:, 0:12, :])
                            nc.sync.dma_start(wo_sb[:, 0:24, :],
                                              wo_r[:, 0:24, :])
                        elif h == 3:
                            nc.sync.dma_start(wo_sb[:, 24:48, :],
                                              wo_r[:, 24:48, :])
                    else:
                        if h == 0:
                            for qq in range(4):
                                nc.sync.dma_start(
                                    wq_sb[:, qq * 36:(qq + 1) * 36, :],
                                    wq_r[:, qq * 36:(qq + 1) * 36, :])
                        elif h == 1:
                            nc.sync.dma_start(memA_t[:], mem_r[:, 0:12, :])
                            nc.sync.dma_start(wo_sb[:, 0:24, :],
                                              wo_r[:, 0:24, :])
                            nc.sync.dma_start(wo_sb[:, 24:48, :],
                                              wo_r[:, 24:48, :])

                kv_tiles = {}

                def kv_proj(h, t, half, dst_ps):
                    base = (h * 2 + half) * 6
                    for i in range(3):
                        nc.tensor.matmul(
                            dst_ps[:],
                            xnT[:, 2 * i:2 * i + 2, t * 128:(t + 1) * 128],
                            wkv_sb[:, base + 2 * i:base + 2 * i + 2, :],
                            start=(i == 0), stop=(i == 2), perf_mode=DR)

                def kv_pair(h, u):
                    # k/v for token tiles 2u, 2u+1; the feature map
                    # (max(z,thr))^1.5 = t*sqrt(t) batched per pair
                    if u == 0:
                        kv_tiles[h] = (
                            kvp.tile([128, NT, HD], FP8, tag="khat",
                                     name=f"khat{h}"),
                            kvp.tile([128, NT, HD], FP8, tag="vaug",
                                     name=f"vaug{h}"),
                        )
                    khat_h, v_h = kv_tiles[h]
                    t16 = scrB.tile([128, 2, HD], BF16, tag="t16")
                    for j in range(2):
                        t = 2 * u + j
                        pkv = ps_kv.tile([128, HD], F32, tag="kv")
                        kv_proj(h, t, 0, pkv)
                        src = pkv
                        if not b_in_zero:
                            pb = scrB.tile([128, HD], F32, tag="scr")
                            nc.vector.tensor_add(
                                pb[:], pkv[:], bkv_sb[:, h * 2, :])
                            src = pb
                        _kp = STRAT["kp1"]
                        _ph = (h + 2 * u + j) % 4
                        _kact = (_kp == 1 or
                                (_kp == 0 and (h + 2 * u + j) % 2 == 0) or
                                (_kp == 3 and _ph != 1) or
                                (_kp == 4 and _ph == 0))
                        if _kact and b_in_zero:
                            nc.scalar.activation(t16[:, j, :], src[:],
                                                 AF.Relu, scale=SQ_PRE)
                        else:
                            with nc.allow_low_precision(reason="fmap"):
                                nc.vector.tensor_scalar(t16[:, j, :], src[:],
                                                        KTHR, SQ_PRE,
                                                        OP.max, OP.mult)
                    s16 = scrB.tile([128, 2, HD], BF16, tag="s16")
                    nc.vector.tensor_scalar(
                        s16[:].bitcast(U16), t16[:].bitcast(U16),
                        1, None, OP.logical_shift_right)
                    kmv = STRAT["kmul"]
                    me = nc.vector if (
                        kmv == 3 or (kmv == 2 and (h + u) % 2 == 1)
                        or (kmv == 1 and (h + u) % 4 == 1)
                        or (kmv == 4 and (h + u) % 4 != 1)) else nc.gpsimd
                    me.tensor_mul(khat_h[:, 2 * u:2 * u + 2, :], t16[:],
                                  s16[:])
                    for j in range(2):
                        t = 2 * u + j
                        pkv = ps_kv.tile([128, HD], F32, tag="kv")
                        kv_proj(h, t, 1, pkv)
                        src = pkv
                        if not b_in_zero:
                            pb = scrB.tile([128, HD], F32, tag="scr")
                            nc.vector.tensor_add(
                                pb[:], pkv[:], bkv_sb[:, h * 2 + 1, :])
                            src = pb
                        _vdve = (STRAT["vcopy"] == 2 or
                                 (STRAT["vcopy"] == 1 and t % 2 == 1))
                        if _vdve:
                            with nc.allow_low_precision(reason="v fp8"):
                                nc.vector.tensor_copy(v_h[:, t, :], src[:])
                        else:
                            nc.scalar.copy(v_h[:, t, :], src[:])

                # ---------------- phase A: LayerNorm + transpose ----------
                with (
                    tc.tile_pool(name="lnp", bufs=2) as lnp,
                    tc.tile_pool(name="ps_tp", bufs=2, space="PSUM") as ps_tp,
                    tc.tile_pool(name="ps_g", bufs=1, space="PSUM") as ps_g,
                ):
                    for t in range(NT):
                        stats = lnp.tile([128, 3, 6], F32, tag="stats")
                        for g in range(3):
                            nc.vector.bn_stats(
                                stats[:, g, :],
                                x_sb[t][:, g * 256:(g + 1) * 256])
                        mv = lnp.tile([128, 2], F32, tag="mv")
                        nc.vector.bn_aggr(mv[:], stats[:])
                        sq = lnp.tile([128, 1], F32, tag="sq")
                        nc.scalar.activation(sq[:], mv[:, 1:2], AF.Sqrt,
                                             bias=lneps_col[:], scale=1.0)
                        rstd = lnp.tile([128, 1], F32, tag="rstd")
                        nc.vector.reciprocal(rstd[:], sq[:])
                        xn = lnp.tile([128, D], BF16, tag="xn")
                        if t % 2 == 0 and STRAT["lnxn"] == 0:
                            with nc.allow_low_precision(reason="xn bf16"):
                                nc.vector.tensor_scalar(xn[:], x_sb[t][:],
                                                        mv[:, 0:1], rstd[:],
                                                        OP.subtract, OP.mult)
                        else:
                            negmr = lnp.tile([128, 1], F32, tag="negmr")
                            nc.vector.tensor_scalar(negmr[:], mv[:, 0:1],
                                                    rstd[:], -1.0,
                                                    OP.mult, OP.mult)
                            nc.scalar.activation(xn[:], x_sb[t][:],
                                                 AF.Identity,
                                                 bias=negmr[:],
                                                 scale=rstd[:])
                        if not ln_trivial:
                            nc.vector.tensor_mul(xn[:], xn[:], lng_b[:])
                            nc.vector.tensor_add(xn[:], xn[:], lnb_b[:])
                        # residue gate logit (DVE accumulate, bf16 xn:
                        # the residue scales the full residual so fp8-xnT
                        # logits are too noisy for it)
                        scr = lnp.tile([128, D], BF16, tag="scr")
                        nc.vector.scalar_tensor_tensor(
                            scr[:], xn[:], 0.0, wres_b[:], OP.add, OP.mult,
                            accum_out=rlogs[:, t:t + 1])
                        # transpose to xnT (fp8)
                        for g in range(2):
                            tp = ps_tp.tile([128, 3, 128], BF16, tag="tp")
                            for c3 in range(3):
                                nc.tensor.transpose(
                                    tp[:, c3, :],
                                    xn[:, (g * 3 + c3) * 128:
                                       (g * 3 + c3 + 1) * 128],
                                    ident[:])
                            dst = xnT[:, g * 3:g * 3 + 3,
                                      t * 128:(t + 1) * 128]
                            nc.scalar.copy(dst, tp[:])
                        # head-0 k/v fills the PE while LN streams
                        if t % 2 == 1:
                            kv_pair(0, t // 2)

                    # gates + residue logits: [rg|wg|res] @ xn via two DR
                    # matmul columns against xnT (rows 0-15 gates, row 16
                    # per-token residue logits)
                    gps = [ps_g.tile([32, 512], F32, tag=f"g{i}",
                                     name=f"gps{i}") for i in range(2)]
                    for i in range(2):
                        for c3 in range(3):
                            nc.tensor.matmul(
                                gps[i][:],
                                wg32_sb[:, 2 * c3:2 * c3 + 2, :],
                                xnT[:, 2 * c3:2 * c3 + 2,
                                    i * 512:(i + 1) * 512],
                                start=(c3 == 0), stop=(c3 == 2),
                                perf_mode=DR)
                    nc.scalar.activation(residue[:], rlogs[:], AF.Sigmoid,
                                         bias=bres_b[:], scale=1.0)
                    glogc = lnp.tile([32, 2], F32, tag="glogc")
                    for i in range(2):
                        nc.vector.tensor_reduce(
                            glogc[:, i:i + 1], gps[i][:],
                            mybir.AxisListType.X, OP.add)
                    glog32 = lnp.tile([32, 1], F32, tag="glog32")
                    nc.vector.tensor_add(glog32[:], glogc[:, 0:1],
                                         glogc[:, 1:2])
                    nc.scalar.activation(gates32[:], glog32[:], AF.Sigmoid,
                                         bias=gb32[:], scale=1.0 / (SW * N))
                    g16 = lnp.tile([32, 1], BF16, tag="g16")
                    nc.vector.tensor_copy(g16[:], gates32[:])
                    nc.sync.dma_start(cc_in_a[CC_HALF:CC_HALF + 8],
                                      g16[8:16, :].opt())
                    nc.sync.dma_start(rg_dram[:], gates32[0:8, :].opt())
                    # E-phase residue scalars (off critical path)
                    nc.vector.tensor_scalar_mul(rdiv[:], residue[:],
                                                1.0 / OUT_DESCALE)
                    nc.vector.tensor_scalar(onemr[:], residue[:], -1.0, 1.0,
                                            OP.mult, OP.add)

                # ------------ phase B: kv + states + q --------------------
                with (
                    tc.tile_pool(name="stgp", bufs=1) as stgp,
                    tc.tile_pool(name="rbpool", bufs=2) as rbpool,
                    tc.tile_pool(name="scrC", bufs=2) as scrC,
                    tc.tile_pool(name="ps_z", bufs=1, space="PSUM") as ps_z,
                    tc.tile_pool(name="ps_st", bufs=1, space="PSUM") as ps_st,
                    tc.tile_pool(name="ps_q", bufs=2, space="PSUM") as ps_q,
                ):
                    rb_tiles = {}

                    def z_group(h):
                        khat_h, _ = kv_tiles[h]
                        zps = ps_z.tile([32, HD], F32, tag="z")
                        for u in range(NT // 2):
                            nc.tensor.matmul(
                                zps[:], ones8t[:, 2 * u:2 * u + 2, :],
                                khat_h[:, 2 * u:2 * u + 2, :],
                                start=(u == 0), stop=(u == NT // 2 - 1),
                                perf_mode=DR)
                        zrec16 = scrB.tile([1, HD], BF16, tag="zrec")
                        with nc.allow_low_precision(reason="1/z row"):
                            nc.vector.reciprocal(zrec16[:], zps[0:1, :])
                        # broadcast 1/z across partitions via DRAM round-trip
                        # (latency hidden by the one-head pipeline lead)
                        nc.sync.dma_start(zr_dram[h * HD:(h + 1) * HD],
                                          zrec16[:].opt())
                        rb = rbpool.tile([128, HD], BF16, tag="rb",
                                         name=f"rb{h}")
                        nc.sync.dma_start(
                            rb[:],
                            zr_dram[h * HD:(h + 1) * HD]
                            .partition_broadcast(128))
                        rb_tiles[h] = rb

                    def rb_state(h):
                        khat_h, v_h = kv_tiles.pop(h)
                        rb = rb_tiles.pop(h)
                        stx = stgp.tile([128, 3, HD], BF16, tag="stg",
                                        name=f"stg{h}")
                        st_eng = nc.vector
                        for ec in range(3):
                            pst = ps_st.tile([128, HD], F32, tag="st")
                            for u in range(NT // 2):
                                nc.tensor.matmul(
                                    pst[:],
                                    v_h[:, 2 * u:2 * u + 2,
                                        ec * 128:(ec + 1) * 128],
                                    khat_h[:, 2 * u:2 * u + 2, :],
                                    start=(u == 0), stop=(u == NT // 2 - 1),
                                    perf_mode=DR)
                            st_eng.scalar_tensor_tensor(
                                stx[:, ec, :], pst[:], 0.0,
                                rb[:], OP.add, OP.mult)
                        cin, slot = ((cc_in_a, h) if h < 4 else
                                     (cc_in_b1, h - 4) if h < 6 else
                                     (cc_in_b2, h - 6))
                        nc.sync.dma_start(
                            cin[slot * HBLK:(slot + 1) * HBLK]
                            .rearrange("(a p m) -> p a m", a=3, p=128),
                            stx[:])
                        if h == 3:
                            do_collective(cc_in_a, cc_out_a, CC_LEN_A)
                        elif h == 5:
                            do_collective(cc_in_b1, cc_out_b1, 2 * HBLK)
                        elif h == 7:
                            do_collective(cc_in_b2, cc_out_b2, 2 * HBLK)

                    absb = None
                    # software pipeline: kv two heads ahead, z one ahead;
                    # four q blocks interleaved per head from h==2
                    for u in range(NT // 2):
                        kv_pair(1, u)
                    z_group(0)
                    ORD = STRAT.get("ord", 0)
                    for h in range(H):
                        def _kv():
                            if h + 2 < H:
                                for u in range(NT // 2):
                                    kv_pair(h + 2, u)
                        def _z():
                            if h + 1 < H:
                                z_group(h + 1)
                        def _st():
                            rb_state(h)
                        def _q():
                            if h >= 2:
                                for k in range(4):
                                    q_block2((h - 2) * 4 + k, scrC, ps_q)
                        orders = {
                            0: (_kv, _z, _st, _q),
                            1: (_kv, _z, _q, _st),
                            2: (_q, _kv, _z, _st),
                            3: (_kv, _q, _z, _st),
                            4: (_z, _kv, _st, _q),
                        }
                        for fn in orders[ORD]:
                            fn()
                        late_loads(h)
                        if h == STRAT.get("dprep", 5):
                            # collective half A landed long ago: get the
                            # D-phase scalars and st read going (DMA/DVE)
                            absb = d_prep()
                            d_read(0, stA_t)

                    # ---- B tail: D-phase heads 0-3 -----------------------
                    for i in range(4):
                        d_piece(0, i, absb, stA_t, mA_t, memA_t, ps_z)

            # ================= phases D (half B) and E =====================
            with tc.tile_pool(name="cp", bufs=1) as cp:
                stB_t = cp.tile([128, 12, 384], BF16)
                mB_t = cp.tile([128, 12, 384], FP8)
                memB_t = cp.tile([128, 12, 384], BF16)
                nc.sync.dma_start(memB_t[:], mem_r[:, 12:24, :])
                d_read(1, stB_t)
                with (
                    tc.tile_pool(name="ps_w2", bufs=2, space="PSUM") as ps_w2,
                ):
                    for hh in range(4):
                        d_piece(1, hh, absb, stB_t, mB_t, memB_t, ps_w2)

                # ============= phase E: output =============================
                with (
                    tc.tile_pool(name="ep", bufs=3) as ep,
                    tc.tile_pool(name="ps_o", bufs=4, space="PSUM") as ps_o,
                ):
                    for t in range(NT):
                        pos = [ps_o.tile([128, HD], F32, tag="o",
                                         name=f"o{t}_{jb}")
                               for jb in range(2)]
                        for e in range(SC // 2):
                            for jb in range(2):
                                nc.tensor.matmul(
                                    pos[jb][:],
                                    qhatT[:, 2 * e:2 * e + 2,
                                          t * 128:(t + 1) * 128],
                                    WpT[:, 2 * e:2 * e + 2,
                                        jb * 384:(jb + 1) * 384],
                                    start=(e == 0), stop=(e == SC // 2 - 1),
                                    perf_mode=DR)
                        u = ep.tile([128, D], F32, tag="u")
                        nc.scalar.mul(u[:], x_sb[t][:], onemr[:, t:t + 1])
                        if not b_out_zero:
                            u2 = ep.tile([128, D], F32, tag="u2")
                            nc.vector.scalar_tensor_tensor(
                                u2[:], bout_b[:], residue[:, t:t + 1], u[:],
                                OP.mult, OP.add)
                            u = u2
                        osb = ep.tile([128, D], F32, tag="osb")
                        for jb in range(2):
                            nc.vector.scalar_tensor_tensor(
                                osb[:, jb * 384:(jb + 1) * 384], pos[jb][:],
                                rdiv[:, t:t + 1],
                                u[:, jb * 384:(jb + 1) * 384],
                                OP.mult, OP.add)
                        nc.sync.dma_start(out_d[t * 128:(t + 1) * 128, :],
                                          osb[:])

    nc.compile()
    return nc


_PROGRAM_CACHE = {}


def _get_program(key):
    if key not in _PROGRAM_CACHE:
        _PROGRAM_CACHE[key] = build_program(*key)
    return _PROGRAM_CACHE[key]


def _pack_weights(w_in, w_out, memory):
    w8 = np.asarray(SW * w_in, NP_FP8)              # [768, 9216]
    # wkv: [p, h, half, i, j, c] -> [128, 96*384]
    wk = np.ascontiguousarray(
        w8[:, S:2 * S].reshape(3, 2, 128, H, HD).transpose(2, 3, 0, 1, 4))
    wv = np.ascontiguousarray(
        w8[:, 2 * S:3 * S].reshape(3, 2, 128, H, HD).transpose(2, 3, 0, 1, 4))
    wkv = np.stack([wk, wv], axis=2)                # [p, h, half, i, j, c]
    wkv = np.ascontiguousarray(wkv).reshape(128, 96 * 384)
    # wq: [p, sc, i, j, m] -> [128, 144*128]
    wq = np.ascontiguousarray(
        w8[:, 0:S].reshape(3, 2, 128, SC, 128).transpose(2, 3, 0, 1, 4))
    wq = wq.reshape(128, 144 * 128)
    # wo: [p, h, jb, ec, d] -> [128, 48*384]
    wo64 = np.asarray(MSC * w_out, NP_FP8)          # [3072, 768]
    wo = np.ascontiguousarray(
        wo64.reshape(H, 3, 128, 2, HD).transpose(2, 0, 3, 1, 4))
    wo = wo.reshape(128, 48 * 384)
    # memT64: [p, h, ec, d] -> [128, 24*384] bf16
    memT = np.ascontiguousarray(MSC * memory.transpose(0, 2, 1))  # [h, e, d]
    memb = np.asarray(memT, NP_BF16).reshape(H, 3, 128, HD)
    memb = np.ascontiguousarray(memb.transpose(2, 0, 1, 3)).reshape(128, 24 * 384)
    return wkv, wq, wo, memb


def kernel(x, memory, ln_g, ln_b, w_in, b_in, w_out, b_out,
           w_rg, b_rg, w_wg, b_wg, w_res, b_res):
    x = np.ascontiguousarray(np.asarray(x, dtype=np.float32))
    memory = np.asarray(memory, dtype=np.float32)
    ln_g = np.asarray(ln_g, dtype=np.float32)
    ln_b = np.asarray(ln_b, dtype=np.float32)
    w_in = np.ascontiguousarray(np.asarray(w_in, dtype=np.float32))
    b_in = np.asarray(b_in, dtype=np.float32)
    w_out = np.asarray(w_out, dtype=np.float32)
    b_out = np.asarray(b_out, dtype=np.float32)
    w_rg = np.asarray(w_rg, dtype=np.float32)
    b_rg = np.asarray(b_rg, dtype=np.float32)
    w_wg = np.asarray(w_wg, dtype=np.float32)
    b_wg = np.asarray(b_wg, dtype=np.float32)
    w_res = np.asarray(w_res, dtype=np.float32)
    b_res = np.asarray(b_res, dtype=np.float32)

    ln_trivial = bool(np.all(ln_g == 1.0) and np.all(ln_b == 0.0))
    b_in_zero = bool(np.all(b_in == 0.0))
    b_out_zero = bool(np.all(b_out == 0.0))

    nc = _get_program((ln_trivial, b_in_zero, b_out_zero))
    wkv, wq, wo, memb = _pack_weights(w_in, w_out, memory)

    wg32 = np.zeros((128, KC, 32), np.float32)
    wg32[:, :, 0:8] = SW * w_rg.reshape(KC, 128, H).transpose(1, 0, 2)
    wg32[:, :, 8:16] = SW * w_wg.reshape(KC, 128, H).transpose(1, 0, 2)
    wg32[:, :, 16] = SW * w_res.reshape(KC, 128).transpose(1, 0)
    gb32 = np.zeros(32, np.float32)
    gb32[0:8] = b_rg
    gb32[8:16] = b_wg
    shared = {
        "wkv": wkv, "wq": wq, "wo": wo, "memT64": memb,
        "wg32": np.asarray(wg32, NP_FP8).reshape(128, KC * 32),
        "gb32": gb32, "b_res": b_res,
        "wres16": np.asarray(w_res[:, 0], NP_BF16),
    }
    if not ln_trivial:
        shared["ln_g"] = ln_g
        shared["ln_b"] = ln_b
    if not b_in_zero:
        bq8 = np.ascontiguousarray(
            (SW * b_in[0:S]).reshape(SC, 128).T.astype(np.float32))
        bkv8 = np.ascontiguousarray(
            (SW * b_in[S:3 * S]).reshape(2, H, HD).transpose(1, 0, 2)
            .reshape(16, HD).astype(np.float32))
        shared["bq8"] = bq8
        shared["bkv8"] = bkv8
    if not b_out_zero:
        shared["b_out"] = b_out

    in_maps = [{"x": x[b], **shared} for b in range(N_CORES)]
    res = run_bass_kernel_spmd(nc, in_maps, list(range(N_CORES)))
    return np.stack([res.results[b]["out"] for b in range(N_CORES)], axis=0)

